# revision 1
# baseline (speedup 1.0000x reference)
"""Single-head attention (InterModalAttention) Bass kernel for 8 TRN2 cores.

Sharding: batch (4) x query-half (2) -> 8 cores. Each core computes K/V for
its batch element (full 2048-seq) and attention for its 1024 queries.

Layout strategy (all matmuls contract over the partition dim):
  - Host pre-transposes x and weights -> xT [d,s], WT [d,e] so no on-chip
    transpose of inputs is needed.
  - qT/kT computed as [e, s] tiles (lhsT=WT tile, rhs=xT tile); bias added
    per-partition during PSUM->SBUF copyback.
  - v computed natural [s, e] (lhsT=xT tile, rhs=WvT tile); bias bv folded
    into the final epilogue (softmax rows sum to 1).
  - scores[i,j] psum accumulated over 8 e-tiles; exp on ACT engine with
    scale=1/32 and accum_out giving row-sums for free.
  - attn tiles PE-transposed (128x128) -> lhsT for out = attnT.T @ v,
    accumulated over 16 j-tiles in PSUM.
  - epilogue: out = psum * (1/rowsum) + bv.
All matmul operands use float32r (full-rate fp32 on the PE at N>=512).
"""
import sys
import numpy as np

for p in ("/opt/trn_rl_repo",):
    if p not in sys.path:
        sys.path.insert(0, p)

B, S, D = 4, 2048, 1024
NQ = 1024          # queries per core
NCORES = 8
P = 128
INV_SQRT_D = 1.0 / 32.0

_CACHE = {}


def build_nc():
    from contextlib import ExitStack
    import concourse.mybir as mybir
    import concourse.tile as tile
    from concourse import bacc
    from concourse.masks import make_identity

    F32 = mybir.dt.float32
    FR = mybir.dt.float32r
    AF = mybir.ActivationFunctionType

    nc = bacc.Bacc("TRN2", debug=False)

    xkvT = nc.dram_tensor("xkvT", (D, S), FR, kind="ExternalInput")
    xqT = nc.dram_tensor("xqT", (D, NQ), FR, kind="ExternalInput")
    wqT = nc.dram_tensor("wqT", (D, D), FR, kind="ExternalInput")
    wkT = nc.dram_tensor("wkT", (D, D), FR, kind="ExternalInput")
    wvT = nc.dram_tensor("wvT", (D, D), FR, kind="ExternalInput")
    bq = nc.dram_tensor("bq", (D,), F32, kind="ExternalInput")
    bk = nc.dram_tensor("bk", (D,), F32, kind="ExternalInput")
    bv = nc.dram_tensor("bv", (D,), F32, kind="ExternalInput")
    out = nc.dram_tensor("out", (NQ, D), F32, kind="ExternalOutput")

    ET = D // P            # 8 e-tiles
    DT = D // P            # 8 d-tiles
    SC = S // 512          # 4 s-chunks
    SB = S // P            # 16 s-blocks (j-tiles)
    IG = NQ // 512         # 2 i-groups
    EC = D // 512          # 2 e-chunks

    with tile.TileContext(nc) as tc, ExitStack() as ctx:
        consts = ctx.enter_context(tc.tile_pool(name="consts", bufs=1))
        ps512 = ctx.enter_context(tc.tile_pool(name="ps512", bufs=2, space="PSUM"))
        outps = ctx.enter_context(tc.tile_pool(name="outps", bufs=2, space="PSUM"))
        tpps = ctx.enter_context(tc.tile_pool(name="tpps", bufs=2, space="PSUM"))
        dram = ctx.enter_context(tc.tile_pool(name="dram", bufs=1, space="DRAM"))

        _eng = [nc.sync, nc.gpsimd, nc.scalar]
        _dmac = [0]
        def dma(out_ap, in_ap):
            e = _eng[_dmac[0] % len(_eng)]
            _dmac[0] += 1
            e.dma_start(out_ap, in_ap)

        # ---- constants ----
        ident_f = consts.tile([P, P], F32)
        make_identity(nc, ident_f)
        ident = consts.tile([P, P], FR)
        nc.gpsimd.dma_start(ident[:], ident_f[:])

        ones_f = consts.tile([1, P], F32)
        nc.gpsimd.memset(ones_f[:], 1.0)
        ones = consts.tile([1, P], FR)
        nc.gpsimd.dma_start(ones[:], ones_f[:])

        bv_sb = consts.tile([1, D], FR)
        nc.gpsimd.dma_start(bv_sb[:], bv[:].rearrange("(one d) -> one d", one=1))
        bq_sb = consts.tile([P, ET], F32)
        nc.sync.dma_start(bq_sb[:], bq[:].rearrange("(t p) -> p t", p=P))
        bk_sb = consts.tile([P, ET], F32)
        nc.sync.dma_start(bk_sb[:], bk[:].rearrange("(t p) -> p t", p=P))

        # bv broadcast to [P, D] via ones.T @ bv (K=1 matmul)
        bv_bcast = consts.tile([P, D], F32)
        for ec in range(EC):
            pstmp = ps512.tile([P, 512], F32, tag="ps512")
            nc.tensor.matmul(pstmp[:], ones[:], bv_sb[:, ec * 512:(ec + 1) * 512],
                             start=True, stop=True)
            nc.any.tensor_copy(bv_bcast[:, ec * 512:(ec + 1) * 512], pstmp[:])

        qT_dram = dram.tile([D, NQ], FR)
        kpool = ctx.enter_context(tc.tile_pool(name="kpool", bufs=1))
        kT = kpool.tile([P, ET, S], FR)      # [e-part, e-tile, j]

        # ---- Phase 1: Q projection (wk prefetched) ----
        wk_ctx = tc.tile_pool(name="wk", bufs=1)
        wkp = wk_ctx.__enter__()
        with tc.tile_pool(name="wq", bufs=1) as wqp, \
             tc.tile_pool(name="xq", bufs=1) as xqp, \
             tc.tile_pool(name="qo", bufs=2) as qop:
            wq_sb = wqp.tile([P, DT, D], FR)
            for dt in range(DT):
                dma(wq_sb[:, dt, :], wqT[dt * P:(dt + 1) * P, :])
            wk_sb = wkp.tile([P, DT, D], FR)
            for dt in range(DT):
                dma(wk_sb[:, dt, :], wkT[dt * P:(dt + 1) * P, :])
            for g in range(IG):
                xq_g = xqp.tile([P, DT, 512], FR, tag="xq")
                for dt in range(DT):
                    dma(xq_g[:, dt, :],
                                      xqT[dt * P:(dt + 1) * P, g * 512:(g + 1) * 512])
                for et in range(ET):
                    psq = ps512.tile([P, 512], F32, tag="ps512")
                    for dt in range(DT):
                        nc.tensor.matmul(psq[:], wq_sb[:, dt, et * P:(et + 1) * P],
                                         xq_g[:, dt, :], start=(dt == 0), stop=(dt == DT - 1))
                    qo = qop.tile([P, 512], FR, tag="qo")
                    nc.vector.tensor_scalar_add(qo[:], psq[:], bq_sb[:, et:et + 1])
                    dma(qT_dram[et * P:(et + 1) * P, g * 512:(g + 1) * 512], qo[:])

        # ---- Phase 2: K projection -> kT resident [e, j] ----
        with tc.tile_pool(name="xk", bufs=2) as xkp:
            for sc in range(SC):
                xk_g = xkp.tile([P, DT, 512], FR, tag="xk")
                for dt in range(DT):
                    dma(xk_g[:, dt, :],
                                      xkvT[dt * P:(dt + 1) * P, sc * 512:(sc + 1) * 512])
                for et in range(ET):
                    psk = ps512.tile([P, 512], F32, tag="ps512")
                    for dt in range(DT):
                        nc.tensor.matmul(psk[:], wk_sb[:, dt, et * P:(et + 1) * P],
                                         xk_g[:, dt, :], start=(dt == 0), stop=(dt == DT - 1))
                    nc.vector.tensor_scalar_add(kT[:, et, sc * 512:(sc + 1) * 512],
                                                psk[:], bk_sb[:, et:et + 1])

        # ---- Phase 3: V projection -> v resident [j, e] (no bias) ----
        wk_ctx.__exit__(None, None, None)
        vpool = ctx.enter_context(tc.tile_pool(name="vpool", bufs=1))
        vN = vpool.tile([P, SB, D], FR)      # [s-part, j-tile, e]
        with tc.tile_pool(name="wv", bufs=1) as wvp, \
             tc.tile_pool(name="xv", bufs=2) as xvp:
            wv_sb = wvp.tile([P, DT, D], FR)
            for dt in range(DT):
                dma(wv_sb[:, dt, :], wvT[dt * P:(dt + 1) * P, :])
            for sb_i in range(SB):
                xv_g = xvp.tile([P, DT, P], FR, tag="xv")
                for dt in range(DT):
                    dma(xv_g[:, dt, :],
                                      xkvT[dt * P:(dt + 1) * P, sb_i * P:(sb_i + 1) * P])
                for ec in range(EC):
                    psv = ps512.tile([P, 512], F32, tag="ps512")
                    for dt in range(DT):
                        nc.tensor.matmul(psv[:], xv_g[:, dt, :],
                                         wv_sb[:, dt, ec * 512:(ec + 1) * 512],
                                         start=(dt == 0), stop=(dt == DT - 1))
                    nc.any.tensor_copy(vN[:, sb_i, ec * 512:(ec + 1) * 512], psv[:])

        # ---- Phase 4: attention ----
        with tc.tile_pool(name="qg", bufs=1) as qgp, \
             tc.tile_pool(name="attn", bufs=3) as attnp, \
             tc.tile_pool(name="attnT", bufs=6) as attnTp, \
             tc.tile_pool(name="epi", bufs=2) as epip:
            for g in range(IG):
                qT_g = qgp.tile([P, ET, 512], FR, tag="qg")
                for et in range(ET):
                    dma(qT_g[:, et, :],
                                      qT_dram[et * P:(et + 1) * P, g * 512:(g + 1) * 512])
                for ib in range(4):
                    i0 = ib * P
                    out_ps = [outps.tile([P, 512], F32, tag=f"outps{ec}", name=f"out_ps{ec}")
                              for ec in range(EC)]
                    rs = epip.tile([P, SC], F32, tag="rs")
                    for jc in range(SC):
                        sc_ps = ps512.tile([P, 512], F32, tag="ps512")
                        for et in range(ET):
                            nc.tensor.matmul(sc_ps[:], qT_g[:, et, i0:i0 + P],
                                             kT[:, et, jc * 512:(jc + 1) * 512],
                                             start=(et == 0), stop=(et == ET - 1))
                        attn = attnp.tile([P, 512], FR, tag="attn")
                        nc.scalar.activation(attn[:], sc_ps[:], AF.Exp,
                                             scale=INV_SQRT_D, accum_out=rs[:, jc:jc + 1])
                        for jt in range(4):
                            jg = jc * 4 + jt
                            tps = tpps.tile([P, P], FR, tag="tps")
                            nc.tensor.transpose(tps[:], attn[:, jt * P:(jt + 1) * P], ident[:])
                            attnT = attnTp.tile([P, P], FR, tag="attnT")
                            nc.any.tensor_copy(attnT[:], tps[:])
                            for ec in range(EC):
                                nc.tensor.matmul(out_ps[ec][:], attnT[:],
                                                 vN[:, jg, ec * 512:(ec + 1) * 512],
                                                 start=(jg == 0), stop=(jg == SB - 1))
                    rsum = epip.tile([P, 1], F32, tag="rsum")
                    nc.vector.tensor_reduce(rsum[:], rs[:], mybir.AxisListType.X,
                                            mybir.AluOpType.add)
                    invs = epip.tile([P, 1], F32, tag="invs")
                    nc.vector.reciprocal(invs[:], rsum[:])
                    out_sb = epip.tile([P, D], F32, tag="out_sb")
                    for ec in range(EC):
                        nc.vector.tensor_scalar_mul(out_sb[:, ec * 512:(ec + 1) * 512],
                                                    out_ps[ec][:], invs[:])
                    nc.vector.tensor_add(out_sb[:], out_sb[:], bv_bcast[:])
                    r0 = g * 512 + i0
                    dma(out[r0:r0 + P, :], out_sb[:])

    nc.compile()
    return nc


def make_in_maps(x, Wq, bq, Wk, bk, Wv, bv):
    x = np.asarray(x, np.float32)
    wqT = np.ascontiguousarray(np.asarray(Wq, np.float32).T)
    wkT = np.ascontiguousarray(np.asarray(Wk, np.float32).T)
    wvT = np.ascontiguousarray(np.asarray(Wv, np.float32).T)
    bq = np.ascontiguousarray(np.asarray(bq, np.float32))
    bk = np.ascontiguousarray(np.asarray(bk, np.float32))
    bv = np.ascontiguousarray(np.asarray(bv, np.float32))
    in_maps = []
    for c in range(NCORES):
        b, h = c // 2, c % 2
        xb = x[b]
        in_maps.append({
            "xkvT": np.ascontiguousarray(xb.T),
            "xqT": np.ascontiguousarray(xb[h * NQ:(h + 1) * NQ].T),
            "wqT": wqT, "wkT": wkT, "wvT": wvT,
            "bq": bq, "bk": bk, "bv": bv,
        })
    return in_maps


def get_nc():
    if "nc" not in _CACHE:
        _CACHE["nc"] = build_nc()
    return _CACHE["nc"]


def kernel(x, Wq, bq, Wk, bk, Wv, bv):
    from concourse.bass_utils import run_bass_kernel_spmd
    nc = get_nc()
    in_maps = make_in_maps(x, Wq, bq, Wk, bk, Wv, bv)
    res = run_bass_kernel_spmd(nc, in_maps, core_ids=list(range(NCORES)))
    out = np.empty((B, S, D), np.float32)
    for c in range(NCORES):
        b, h = c // 2, c % 2
        out[b, h * NQ:(h + 1) * NQ] = res.results[c]["out"]
    return out



# revision 7
# speedup vs baseline: 1.2718x; 1.2718x over previous
"""Single-head attention (InterModalAttention) Bass kernel for 8 TRN2 cores.

Sharding: batch (4) x query-half (2) -> 8 cores. Each core computes K/V for
its batch element (full 2048-seq) and attention for its 1024 queries.

v2 design (vs baseline):
  - Single pass over x: K, Q, V projections all consume the same x chunk.
    Host supplies x^T in bf16 with "mine-first" column order (own query half
    first) so Q reads columns 0..1023; attention is invariant to key order.
  - Weights/x/v/attn in bf16 (PE rate identical, DMA+SBUF halved); logits
    path (qT, kT) kept fp32r for precision.
  - Scores computed TRANSPOSED: scT[j,i] = kT_tile.T @ qT_tile, so the exp'd
    tile is directly the lhsT of the output matmul -- no PE transposes, no
    PSUM->SBUF attn copies.
  - Softmax row-sums via ones-stationary matmul [1,512] accumulated over
    j-tiles in PSUM; transposed to per-partition layout with one tiny DMA.
  - qT stays in SBUF (no DRAM roundtrip); DMA interleaved at dt-granularity
    so the PE starts within ~2us.
"""
import sys
import numpy as np

for p in ("/opt/trn_rl_repo",):
    if p not in sys.path:
        sys.path.insert(0, p)

B, S, D = 4, 2048, 1024
NQ = 1024          # queries per core
NCORES = 8
P = 128
INV_SQRT_D = 1.0 / 32.0

_CACHE = {}


def build_nc():
    from contextlib import ExitStack
    import concourse.mybir as mybir
    import concourse.tile as tile
    from concourse import bacc

    F32 = mybir.dt.float32
    FR = mybir.dt.float32r
    BF = mybir.dt.bfloat16
    AF = mybir.ActivationFunctionType

    nc = bacc.Bacc("TRN2", debug=False)

    xT = nc.dram_tensor("xT", (D, S), BF, kind="ExternalInput")    # mine-first cols
    wqT = nc.dram_tensor("wqT", (D, D), BF, kind="ExternalInput")
    wkT = nc.dram_tensor("wkT", (D, D), BF, kind="ExternalInput")
    wvT = nc.dram_tensor("wvT", (D, D), BF, kind="ExternalInput")
    bq = nc.dram_tensor("bq", (D,), F32, kind="ExternalInput")
    bk = nc.dram_tensor("bk", (D,), F32, kind="ExternalInput")
    bv = nc.dram_tensor("bv", (D,), F32, kind="ExternalInput")
    out = nc.dram_tensor("out", (NQ, D), F32, kind="ExternalOutput")

    ET = D // P            # 8 e-tiles
    DT = D // P            # 8 d-tiles
    SC = S // 512          # 4 s-chunks
    SB = S // P            # 16 s-blocks (j-tiles)
    IG = NQ // 512         # 2 i-chunks
    EC = D // 512          # 2 e-chunks

    with tile.TileContext(nc) as tc, ExitStack() as ctx:
        consts = ctx.enter_context(tc.tile_pool(name="consts", bufs=1))

        _eng = [nc.sync, nc.gpsimd, nc.scalar]
        _dmac = [0]
        def dma(out_ap, in_ap):
            e = _eng[_dmac[0] % len(_eng)]
            _dmac[0] += 1
            e.dma_start(out_ap, in_ap)

        # ---- constants ----
        ones_f = consts.tile([1, P], F32)
        nc.gpsimd.memset(ones_f[:], 1.0)
        ones = consts.tile([1, P], FR)
        nc.gpsimd.dma_start(ones[:], ones_f[:])
        onesb = consts.tile([P, 1], BF)
        nc.gpsimd.memset(onesb[:], 1.0)

        bv_sb = consts.tile([1, D], FR)
        nc.gpsimd.dma_start(bv_sb[:], bv[:].rearrange("(one d) -> one d", one=1))
        bq_sb = consts.tile([P, ET], F32)
        nc.sync.dma_start(bq_sb[:], bq[:].rearrange("(t p) -> p t", p=P))
        bk_sb = consts.tile([P, ET], F32)
        nc.sync.dma_start(bk_sb[:], bk[:].rearrange("(t p) -> p t", p=P))

        # resident tensors
        kqv = ctx.enter_context(tc.tile_pool(name="kqv", bufs=1))
        kT = kqv.tile([P, ET, S], FR)       # [d-part, e-tile, j]
        qT = kqv.tile([P, ET, NQ], FR)      # [d-part, e-tile, i]
        vN = kqv.tile([P, SB, D], BF)       # [j-part, j-tile, e]

        # bv broadcast to [P, D] via ones.T @ bv (K=1 matmul)
        bv_bcast = consts.tile([P, D], F32)
        with tc.tile_pool(name="bvps", bufs=1, space="PSUM") as bvps:
            for ec in range(EC):
                pstmp = bvps.tile([P, 512], F32, tag="bvps")
                nc.tensor.matmul(pstmp[:], ones[:], bv_sb[:, ec * 512:(ec + 1) * 512],
                                 start=True, stop=True)
                nc.any.tensor_copy(bv_bcast[:, ec * 512:(ec + 1) * 512], pstmp[:])

        # ---- Phase 1: projections, single pass over x ----
        with tc.tile_pool(name="w", bufs=1) as wp, \
             tc.tile_pool(name="xc", bufs=2) as xcp, \
             tc.tile_pool(name="pp", bufs=3, space="PSUM") as pp:
            wk_sb = wp.tile([P, DT, D], BF)
            wq_sb = wp.tile([P, DT, D], BF)
            wv_sb = wp.tile([P, DT, D], BF)
            # prefetch in consumption order: wk+x0 first, then wq, wv, x1..x3
            xc = []
            for sc in range(SC):
                xc.append(xcp.tile([P, DT, 512], BF, tag="xc", name=f"xc{sc}"))
            for dt in range(DT):
                dma(wk_sb[:, dt, :], wkT[dt * P:(dt + 1) * P, :])
                dma(xc[0][:, dt, :], xT[dt * P:(dt + 1) * P, 0:512])
            for dt in range(DT):
                dma(wq_sb[:, dt, :], wqT[dt * P:(dt + 1) * P, :])
            for dt in range(DT):
                dma(wv_sb[:, dt, :], wvT[dt * P:(dt + 1) * P, :])
            for sc in range(1, SC):
                for dt in range(DT):
                    dma(xc[sc][:, dt, :],
                        xT[dt * P:(dt + 1) * P, sc * 512:(sc + 1) * 512])

            for sc in range(SC):
                x_g = xc[sc]
                # K projection -> kT[:, :, sc-chunk]
                for et in range(ET):
                    psk = pp.tile([P, 512], F32, tag="pp")
                    for dt in range(DT):
                        nc.tensor.matmul(psk[:], wk_sb[:, dt, et * P:(et + 1) * P],
                                         x_g[:, dt, :], start=(dt == 0),
                                         stop=(dt == DT - 1))
                    nc.vector.tensor_scalar_add(kT[:, et, sc * 512:(sc + 1) * 512],
                                                psk[:], bk_sb[:, et:et + 1])
                # Q projection (own half = first 2 chunks)
                if sc < IG:
                    for et in range(ET):
                        psq = pp.tile([P, 512], F32, tag="pp")
                        for dt in range(DT):
                            nc.tensor.matmul(psq[:], wq_sb[:, dt, et * P:(et + 1) * P],
                                             x_g[:, dt, :], start=(dt == 0),
                                             stop=(dt == DT - 1))
                        nc.vector.tensor_scalar_add(qT[:, et, sc * 512:(sc + 1) * 512],
                                                    psq[:], bq_sb[:, et:et + 1])
                # V projection -> vN rows for this chunk (no bias; folded in epilogue)
                for sb_i in range(4):
                    jg = sc * 4 + sb_i
                    for ec in range(EC):
                        psv = pp.tile([P, 512], F32, tag="pp")
                        for dt in range(DT):
                            nc.tensor.matmul(psv[:],
                                             x_g[:, dt, sb_i * P:(sb_i + 1) * P],
                                             wv_sb[:, dt, ec * 512:(ec + 1) * 512],
                                             start=(dt == 0), stop=(dt == DT - 1))
                        nc.any.tensor_copy(vN[:, jg, ec * 512:(ec + 1) * 512], psv[:])

        # ---- Phase 2: attention ----
        with tc.tile_pool(name="attn", bufs=1) as attnp, \
             tc.tile_pool(name="scps", bufs=2, space="PSUM") as scps, \
             tc.tile_pool(name="rsps", bufs=2, space="PSUM") as rsps, \
             tc.tile_pool(name="outps", bufs=2, space="PSUM") as outps, \
             tc.tile_pool(name="rsdram", bufs=2, space="DRAM") as rsdram, \
             tc.tile_pool(name="epi", bufs=2) as epip:
            attnT = attnp.tile([P, SB, 512], BF)     # [j-part, j-tile, i] for one chunk
            for g in range(IG):
                # scores (transposed) + exp + rowsum
                rs = rsps.tile([1, 512], F32, tag="rs", name=f"rs{g}")
                for jt in range(SB):
                    sc_ps = scps.tile([P, 512], F32, tag="scps")
                    for et in range(ET):
                        nc.tensor.matmul(sc_ps[:],
                                         kT[:, et, jt * P:(jt + 1) * P],
                                         qT[:, et, g * 512:(g + 1) * 512],
                                         start=(et == 0), stop=(et == ET - 1))
                    nc.scalar.activation(attnT[:, jt, :], sc_ps[:], AF.Exp,
                                         scale=INV_SQRT_D)
                    nc.tensor.matmul(rs[:], onesb[:], attnT[:, jt, :],
                                     start=(jt == 0), stop=(jt == SB - 1))
                # rowsums -> per-partition layout [128, 4] via DRAM bounce
                # (SBUF partition data can't be re-partitioned by an AP alone)
                rs_sb = epip.tile([1, 512], F32, tag="rs_sb")
                nc.vector.tensor_copy(rs_sb[:], rs[:])
                rs_d = rsdram.tile([1, 512], F32, tag="rs_d")
                nc.sync.dma_start(rs_d[:], rs_sb[:])
                rsT = epip.tile([P, 4], F32, tag="rsT")
                nc.sync.dma_start(
                    rsT[:], rs_d[:].rearrange("one (b p) -> p (one b)", p=P))
                invs = epip.tile([P, 4], F32, tag="invs")
                nc.vector.reciprocal(invs[:], rsT[:])
                # output matmuls + epilogue per 128-query block
                for ib in range(4):
                    i0 = ib * P
                    ops = [outps.tile([P, 512], F32, tag=f"outps{ec}",
                                      name=f"ops{g}_{ib}_{ec}") for ec in range(EC)]
                    for jt in range(SB):
                        for ec in range(EC):
                            nc.tensor.matmul(ops[ec][:],
                                             attnT[:, jt, i0:i0 + P],
                                             vN[:, jt, ec * 512:(ec + 1) * 512],
                                             start=(jt == 0), stop=(jt == SB - 1))
                    out_sb = epip.tile([P, D], F32, tag="out_sb")
                    for ec in range(EC):
                        nc.vector.tensor_scalar_mul(out_sb[:, ec * 512:(ec + 1) * 512],
                                                    ops[ec][:], invs[:, ib:ib + 1])
                    nc.vector.tensor_add(out_sb[:], out_sb[:], bv_bcast[:])
                    r0 = g * 512 + i0
                    dma(out[r0:r0 + P, :], out_sb[:])

    nc.compile()
    return nc


def make_in_maps(x, Wq, bq, Wk, bk, Wv, bv):
    import ml_dtypes
    BF = ml_dtypes.bfloat16
    x = np.asarray(x, np.float32)
    wqT = np.ascontiguousarray(np.asarray(Wq, np.float32).T.astype(BF))
    wkT = np.ascontiguousarray(np.asarray(Wk, np.float32).T.astype(BF))
    wvT = np.ascontiguousarray(np.asarray(Wv, np.float32).T.astype(BF))
    bq = np.ascontiguousarray(np.asarray(bq, np.float32))
    bk = np.ascontiguousarray(np.asarray(bk, np.float32))
    bv = np.ascontiguousarray(np.asarray(bv, np.float32))
    in_maps = []
    for c in range(NCORES):
        b, h = c // 2, c % 2
        xb = x[b]
        # mine-first row order, then transpose -> [D, S]
        x_mf = np.concatenate([xb[h * NQ:(h + 1) * NQ], xb[(1 - h) * NQ:(2 - h) * NQ]], 0)
        in_maps.append({
            "xT": np.ascontiguousarray(x_mf.T.astype(BF)),
            "wqT": wqT, "wkT": wkT, "wvT": wvT,
            "bq": bq, "bk": bk, "bv": bv,
        })
    return in_maps


def get_nc():
    if "nc" not in _CACHE:
        _CACHE["nc"] = build_nc()
    return _CACHE["nc"]


def kernel(x, Wq, bq, Wk, bk, Wv, bv):
    from concourse.bass_utils import run_bass_kernel_spmd
    nc = get_nc()
    in_maps = make_in_maps(x, Wq, bq, Wk, bk, Wv, bv)
    res = run_bass_kernel_spmd(nc, in_maps, core_ids=list(range(NCORES)))
    out = np.empty((B, S, D), np.float32)
    for c in range(NCORES):
        b, h = c // 2, c % 2
        out[b, h * NQ:(h + 1) * NQ] = res.results[c]["out"]
    return out


# revision 9
# speedup vs baseline: 1.3240x; 1.0410x over previous
"""Single-head attention (InterModalAttention) Bass kernel for 8 TRN2 cores.

Sharding: batch (4) x query-half (2) -> 8 cores. Each core computes K/V for
its batch element (full 2048-seq) and attention for its 1024 queries.

v2 design (vs baseline):
  - Single pass over x: K, Q, V projections all consume the same x chunk.
    Host supplies x^T in bf16 with "mine-first" column order (own query half
    first) so Q reads columns 0..1023; attention is invariant to key order.
  - Weights/x/v/attn in bf16 (PE rate identical, DMA+SBUF halved); logits
    path (qT, kT) kept fp32r for precision.
  - Scores computed TRANSPOSED: scT[j,i] = kT_tile.T @ qT_tile, so the exp'd
    tile is directly the lhsT of the output matmul -- no PE transposes, no
    PSUM->SBUF attn copies.
  - Softmax row-sums via ones-stationary matmul [1,512] accumulated over
    j-tiles in PSUM; transposed to per-partition layout with one tiny DMA.
  - qT stays in SBUF (no DRAM roundtrip); DMA interleaved at dt-granularity
    so the PE starts within ~2us.
"""
import sys
import numpy as np

for p in ("/opt/trn_rl_repo",):
    if p not in sys.path:
        sys.path.insert(0, p)

B, S, D = 4, 2048, 1024
NQ = 1024          # queries per core
NCORES = 8
P = 128
INV_SQRT_D = 1.0 / 32.0

_CACHE = {}


def build_nc():
    from contextlib import ExitStack
    import concourse.mybir as mybir
    import concourse.tile as tile
    from concourse import bacc

    F32 = mybir.dt.float32
    FR = mybir.dt.float32r
    BF = mybir.dt.bfloat16
    AF = mybir.ActivationFunctionType

    nc = bacc.Bacc("TRN2", debug=False)

    xT = nc.dram_tensor("xT", (D, S), BF, kind="ExternalInput")    # mine-first cols
    wqT = nc.dram_tensor("wqT", (D, D), BF, kind="ExternalInput")
    wkT = nc.dram_tensor("wkT", (D, D), BF, kind="ExternalInput")
    wvT = nc.dram_tensor("wvT", (D, D), BF, kind="ExternalInput")
    bq = nc.dram_tensor("bq", (D,), F32, kind="ExternalInput")
    bk = nc.dram_tensor("bk", (D,), F32, kind="ExternalInput")
    bv = nc.dram_tensor("bv", (D,), F32, kind="ExternalInput")
    out = nc.dram_tensor("out", (NQ, D), F32, kind="ExternalOutput")

    ET = D // P            # 8 e-tiles
    DT = D // P            # 8 d-tiles
    SC = S // 512          # 4 s-chunks
    SB = S // P            # 16 s-blocks (j-tiles)
    IG = NQ // 512         # 2 i-chunks
    EC = D // 512          # 2 e-chunks

    with tile.TileContext(nc) as tc, ExitStack() as ctx:
        consts = ctx.enter_context(tc.tile_pool(name="consts", bufs=1))

        _eng = [nc.sync, nc.gpsimd, nc.scalar]
        _dmac = [0]
        def dma(out_ap, in_ap):
            e = _eng[_dmac[0] % len(_eng)]
            _dmac[0] += 1
            e.dma_start(out_ap, in_ap)

        # resident tensors
        kqv = ctx.enter_context(tc.tile_pool(name="kqv", bufs=1))
        kT = kqv.tile([P, ET, S], BF)       # [d-part, e-tile, j]
        qT = kqv.tile([P, ET, NQ], BF)      # [d-part, e-tile, i]
        vN = kqv.tile([P, SB, D], BF)       # [j-part, j-tile, e]
        bv_bcast = consts.tile([P, D], F32)

        # ---- Phase 1: projections, single pass over x ----
        with tc.tile_pool(name="w", bufs=1) as wp, \
             tc.tile_pool(name="xc", bufs=2) as xcp:
            wk_sb = wp.tile([P, DT, D], BF)
            wq_sb = wp.tile([P, DT, D], BF)
            wv_sb = wp.tile([P, DT, D], BF)
            # DMA issue in consumption order: wk+x0 first, then wq, wv, x1..x3
            xc = []
            for sc in range(SC):
                xc.append(xcp.tile([P, DT, 512], BF, tag="xc", name=f"xc{sc}"))
            for dt in range(DT):
                dma(wk_sb[:, dt, :], wkT[dt * P:(dt + 1) * P, :])
                dma(xc[0][:, dt, :], xT[dt * P:(dt + 1) * P, 0:512])

            # constants (issued after the critical-path DMAs)
            ones_f = consts.tile([1, P], F32)
            nc.vector.memset(ones_f[:], 1.0)
            ones = consts.tile([1, P], FR)
            nc.gpsimd.dma_start(ones[:], ones_f[:])
            onesb = consts.tile([P, 1], BF)
            nc.vector.memset(onesb[:], 1.0)
            bv_sb = consts.tile([1, D], FR)
            nc.gpsimd.dma_start(bv_sb[:], bv[:].rearrange("(one d) -> one d", one=1))
            bq_sb = consts.tile([P, ET], F32)
            nc.gpsimd.dma_start(bq_sb[:], bq[:].rearrange("(t p) -> p t", p=P))
            bk_sb = consts.tile([P, ET], F32)
            nc.gpsimd.dma_start(bk_sb[:], bk[:].rearrange("(t p) -> p t", p=P))

            for dt in range(DT):
                dma(wq_sb[:, dt, :], wqT[dt * P:(dt + 1) * P, :])
            for dt in range(DT):
                dma(wv_sb[:, dt, :], wvT[dt * P:(dt + 1) * P, :])
            for sc in range(1, SC):
                for dt in range(DT):
                    dma(xc[sc][:, dt, :],
                        xT[dt * P:(dt + 1) * P, sc * 512:(sc + 1) * 512])

            # chunk-0 K projection dt-outer/et-inner: the first 8 matmuls only
            # need wk[dt0]+x0[dt0], so the PE starts as soon as ~384KB landed.
            with tc.tile_pool(name="p8", bufs=8, space="PSUM") as p8:
                psk0 = [p8.tile([P, 512], F32, tag="p8", name=f"psk0_{et}")
                        for et in range(ET)]
                for dt in range(DT):
                    for et in range(ET):
                        nc.tensor.matmul(psk0[et][:],
                                         wk_sb[:, dt, et * P:(et + 1) * P],
                                         xc[0][:, dt, :], start=(dt == 0),
                                         stop=(dt == DT - 1))
                for et in range(ET):
                    nc.vector.tensor_scalar_add(kT[:, et, 0:512],
                                                psk0[et][:], bk_sb[:, et:et + 1])

            with tc.tile_pool(name="pp", bufs=3, space="PSUM") as pp:
                # bv broadcast to [P, D] via ones.T @ bv (K=1 matmul)
                for ec in range(EC):
                    pstmp = pp.tile([P, 512], F32, tag="pp")
                    nc.tensor.matmul(pstmp[:], ones[:],
                                     bv_sb[:, ec * 512:(ec + 1) * 512],
                                     start=True, stop=True)
                    nc.any.tensor_copy(bv_bcast[:, ec * 512:(ec + 1) * 512], pstmp[:])

                for sc in range(SC):
                    x_g = xc[sc]
                    # K projection -> kT[:, :, sc-chunk] (chunk 0 done above)
                    if sc > 0:
                        for et in range(ET):
                            psk = pp.tile([P, 512], F32, tag="pp")
                            for dt in range(DT):
                                nc.tensor.matmul(psk[:],
                                                 wk_sb[:, dt, et * P:(et + 1) * P],
                                                 x_g[:, dt, :], start=(dt == 0),
                                                 stop=(dt == DT - 1))
                            nc.vector.tensor_scalar_add(
                                kT[:, et, sc * 512:(sc + 1) * 512],
                                psk[:], bk_sb[:, et:et + 1])
                    # Q projection (own half = first 2 chunks)
                    if sc < IG:
                        for et in range(ET):
                            psq = pp.tile([P, 512], F32, tag="pp")
                            for dt in range(DT):
                                nc.tensor.matmul(psq[:],
                                                 wq_sb[:, dt, et * P:(et + 1) * P],
                                                 x_g[:, dt, :], start=(dt == 0),
                                                 stop=(dt == DT - 1))
                            nc.vector.tensor_scalar_add(
                                qT[:, et, sc * 512:(sc + 1) * 512],
                                psq[:], bq_sb[:, et:et + 1])
                    # V projection -> vN rows (no bias; folded in epilogue)
                    for sb_i in range(4):
                        jg = sc * 4 + sb_i
                        for ec in range(EC):
                            psv = pp.tile([P, 512], F32, tag="pp")
                            for dt in range(DT):
                                nc.tensor.matmul(psv[:],
                                                 x_g[:, dt, sb_i * P:(sb_i + 1) * P],
                                                 wv_sb[:, dt, ec * 512:(ec + 1) * 512],
                                                 start=(dt == 0), stop=(dt == DT - 1))
                            nc.any.tensor_copy(vN[:, jg, ec * 512:(ec + 1) * 512],
                                               psv[:])

        # ---- Phase 2: attention ----
        with tc.tile_pool(name="attn", bufs=1) as attnp, \
             tc.tile_pool(name="scps", bufs=2, space="PSUM") as scps, \
             tc.tile_pool(name="rsps", bufs=2, space="PSUM") as rsps, \
             tc.tile_pool(name="outps", bufs=2, space="PSUM") as outps, \
             tc.tile_pool(name="rsdram", bufs=2, space="DRAM") as rsdram, \
             tc.tile_pool(name="epi", bufs=2) as epip:
            attnT = attnp.tile([P, SB, 512], BF)     # [j-part, j-tile, i] for one chunk
            for g in range(IG):
                # scores (transposed) + exp + rowsum
                rs = rsps.tile([1, 512], F32, tag="rs", name=f"rs{g}")
                for jt in range(SB):
                    sc_ps = scps.tile([P, 512], F32, tag="scps")
                    for et in range(ET):
                        nc.tensor.matmul(sc_ps[:],
                                         kT[:, et, jt * P:(jt + 1) * P],
                                         qT[:, et, g * 512:(g + 1) * 512],
                                         start=(et == 0), stop=(et == ET - 1))
                    nc.scalar.activation(attnT[:, jt, :], sc_ps[:], AF.Exp,
                                         scale=INV_SQRT_D)
                    nc.tensor.matmul(rs[:], onesb[:], attnT[:, jt, :],
                                     start=(jt == 0), stop=(jt == SB - 1))
                # rowsums -> per-partition layout [128, 4] via DRAM bounce
                # (SBUF partition data can't be re-partitioned by an AP alone)
                rs_sb = epip.tile([1, 512], F32, tag="rs_sb")
                nc.vector.tensor_copy(rs_sb[:], rs[:])
                rs_d = rsdram.tile([1, 512], F32, tag="rs_d")
                nc.sync.dma_start(rs_d[:], rs_sb[:])
                rsT = epip.tile([P, 4], F32, tag="rsT")
                nc.sync.dma_start(
                    rsT[:], rs_d[:].rearrange("one (b p) -> p (one b)", p=P))
                invs = epip.tile([P, 4], F32, tag="invs")
                nc.vector.reciprocal(invs[:], rsT[:])
                # output matmuls + epilogue per 128-query block
                for ib in range(4):
                    i0 = ib * P
                    ops = [outps.tile([P, 512], F32, tag=f"outps{ec}",
                                      name=f"ops{g}_{ib}_{ec}") for ec in range(EC)]
                    for jt in range(SB):
                        for ec in range(EC):
                            nc.tensor.matmul(ops[ec][:],
                                             attnT[:, jt, i0:i0 + P],
                                             vN[:, jt, ec * 512:(ec + 1) * 512],
                                             start=(jt == 0), stop=(jt == SB - 1))
                    out_sb = epip.tile([P, D], F32, tag="out_sb")
                    for ec in range(EC):
                        nc.vector.tensor_scalar_mul(out_sb[:, ec * 512:(ec + 1) * 512],
                                                    ops[ec][:], invs[:, ib:ib + 1])
                    nc.vector.tensor_add(out_sb[:], out_sb[:], bv_bcast[:])
                    r0 = g * 512 + i0
                    dma(out[r0:r0 + P, :], out_sb[:])

    nc.compile()
    return nc


def make_in_maps(x, Wq, bq, Wk, bk, Wv, bv):
    import ml_dtypes
    BF = ml_dtypes.bfloat16
    x = np.asarray(x, np.float32)
    wqT = np.ascontiguousarray(np.asarray(Wq, np.float32).T.astype(BF))
    wkT = np.ascontiguousarray(np.asarray(Wk, np.float32).T.astype(BF))
    wvT = np.ascontiguousarray(np.asarray(Wv, np.float32).T.astype(BF))
    bq = np.ascontiguousarray(np.asarray(bq, np.float32))
    bk = np.ascontiguousarray(np.asarray(bk, np.float32))
    bv = np.ascontiguousarray(np.asarray(bv, np.float32))
    in_maps = []
    for c in range(NCORES):
        b, h = c // 2, c % 2
        xb = x[b]
        # mine-first row order, then transpose -> [D, S]
        x_mf = np.concatenate([xb[h * NQ:(h + 1) * NQ], xb[(1 - h) * NQ:(2 - h) * NQ]], 0)
        in_maps.append({
            "xT": np.ascontiguousarray(x_mf.T.astype(BF)),
            "wqT": wqT, "wkT": wkT, "wvT": wvT,
            "bq": bq, "bk": bk, "bv": bv,
        })
    return in_maps


def get_nc():
    if "nc" not in _CACHE:
        _CACHE["nc"] = build_nc()
    return _CACHE["nc"]


def kernel(x, Wq, bq, Wk, bk, Wv, bv):
    from concourse.bass_utils import run_bass_kernel_spmd
    nc = get_nc()
    in_maps = make_in_maps(x, Wq, bq, Wk, bk, Wv, bv)
    res = run_bass_kernel_spmd(nc, in_maps, core_ids=list(range(NCORES)))
    out = np.empty((B, S, D), np.float32)
    for c in range(NCORES):
        b, h = c // 2, c % 2
        out[b, h * NQ:(h + 1) * NQ] = res.results[c]["out"]
    return out


# revision 10
# speedup vs baseline: 1.5063x; 1.1377x over previous
"""Single-head attention (InterModalAttention) Bass kernel for 8 TRN2 cores.

Sharding: batch (4) x query/kv-half (2) -> 8 cores. Core (2b+h) projects
Q/K/V only for its OWN 1024 rows of batch b; the K and V halves are then
exchanged between the pair (2b, 2b+1) with an HBM AllGather so each core
holds the full 2048-key K/V in absolute order. This halves the projection
FLOPs vs computing K/V redundantly per core.

Other design points:
  - bf16 everywhere on the PE (same PE rate as fp32r, half DMA/SBUF);
    accumulation stays fp32 in PSUM.
  - Scores computed TRANSPOSED: scT[j,i] = kT_tile.T @ qT_tile, so the exp'd
    tile is directly the lhsT of the output matmul -- no PE transposes.
  - Softmax row-sums via ones-stationary matmul [1,512] accumulated over
    j-tiles in PSUM; moved to per-partition layout with a DRAM bounce.
  - Single pass over x (2MB per core); first K chunk runs dt-outer/et-inner
    across 8 PSUM banks so the PE starts after ~384KB of DMA.
  - AllGather overlaps with the V/Q projections; readback overlaps with Q.
"""
import sys
import numpy as np

for p in ("/opt/trn_rl_repo",):
    if p not in sys.path:
        sys.path.insert(0, p)

B, S, D = 4, 2048, 1024
NQ = 1024          # queries (and owned keys) per core
NCORES = 8
P = 128
INV_SQRT_D = 1.0 / 32.0
PAIRS = [[0, 1], [2, 3], [4, 5], [6, 7]]

_CACHE = {}


def build_nc():
    from contextlib import ExitStack
    import concourse.mybir as mybir
    import concourse.tile as tile
    from concourse import bacc

    F32 = mybir.dt.float32
    FR = mybir.dt.float32r
    BF = mybir.dt.bfloat16
    AF = mybir.ActivationFunctionType

    nc = bacc.Bacc("TRN2", debug=False, num_devices=NCORES)

    xT = nc.dram_tensor("xT", (D, NQ), BF, kind="ExternalInput")   # own rows only
    wqT = nc.dram_tensor("wqT", (D, D), BF, kind="ExternalInput")
    wkT = nc.dram_tensor("wkT", (D, D), BF, kind="ExternalInput")
    wvT = nc.dram_tensor("wvT", (D, D), BF, kind="ExternalInput")
    bq = nc.dram_tensor("bq", (D,), F32, kind="ExternalInput")
    bk = nc.dram_tensor("bk", (D,), F32, kind="ExternalInput")
    bv = nc.dram_tensor("bv", (D,), F32, kind="ExternalInput")
    out = nc.dram_tensor("out", (NQ, D), F32, kind="ExternalOutput")

    ET = D // P            # 8 e-tiles
    DT = D // P            # 8 d-tiles
    HC = NQ // 512         # 2 s-chunks over own half
    SB = S // P            # 16 j-tiles (full seq)
    HB = NQ // P           # 8 j-tiles (own half)
    IG = NQ // 512         # 2 i-chunks
    EC = D // 512          # 2 e-chunks

    with tile.TileContext(nc) as tc, ExitStack() as ctx:
        consts = ctx.enter_context(tc.tile_pool(name="consts", bufs=1))

        _eng = [nc.sync, nc.gpsimd, nc.scalar]
        _dmac = [0]
        def dma(out_ap, in_ap):
            e = _eng[_dmac[0] % len(_eng)]
            _dmac[0] += 1
            e.dma_start(out_ap, in_ap)

        # resident tensors
        kqv = ctx.enter_context(tc.tile_pool(name="kqv", bufs=1))
        kT = kqv.tile([P, ET, S], BF)       # [d-part, e-tile, j] full seq (gathered)
        qT = kqv.tile([P, ET, NQ], BF)      # [d-part, e-tile, i]
        vN = kqv.tile([P, SB, D], BF)       # [j-part, j-tile, e] full seq (gathered)
        bv_bcast = consts.tile([P, D], F32)

        # DRAM bounce buffers for the pairwise K/V AllGather
        ccd = ctx.enter_context(tc.tile_pool(name="ccd", bufs=1, space="DRAM"))
        kb_in = ccd.tile([P, ET, NQ], BF)
        kb_out = ccd.tile([2, P, ET, NQ], BF)
        vb_in = ccd.tile([P, HB, D], BF)
        vb_out = ccd.tile([2, P, HB, D], BF)

        # ---- Phase 1: projections over own half, single pass over x ----
        with tc.tile_pool(name="w", bufs=1) as wp, \
             tc.tile_pool(name="xc", bufs=2) as xcp, \
             tc.tile_pool(name="kv_own", bufs=1) as ownp:
            kown = ownp.tile([P, ET, NQ], BF)   # [d-part, e-tile, own j]
            vown = ownp.tile([P, HB, D], BF)    # [own j-part, j-tile, e]
            wk_sb = wp.tile([P, DT, D], BF)
            wq_sb = wp.tile([P, DT, D], BF)
            wv_sb = wp.tile([P, DT, D], BF)
            # DMA issue in consumption order: wk+x first, then wv, wq
            xc = []
            for hc in range(HC):
                xc.append(xcp.tile([P, DT, 512], BF, tag="xc", name=f"xc{hc}"))
            for dt in range(DT):
                dma(wk_sb[:, dt, :], wkT[dt * P:(dt + 1) * P, :])
                dma(xc[0][:, dt, :], xT[dt * P:(dt + 1) * P, 0:512])

            # constants (issued after the critical-path DMAs)
            ones_f = consts.tile([1, P], F32)
            nc.vector.memset(ones_f[:], 1.0)
            ones = consts.tile([1, P], FR)
            nc.gpsimd.dma_start(ones[:], ones_f[:])
            onesb = consts.tile([P, 1], BF)
            nc.vector.memset(onesb[:], 1.0)
            bv_sb = consts.tile([1, D], FR)
            nc.gpsimd.dma_start(bv_sb[:], bv[:].rearrange("(one d) -> one d", one=1))
            bq_sb = consts.tile([P, ET], F32)
            nc.gpsimd.dma_start(bq_sb[:], bq[:].rearrange("(t p) -> p t", p=P))
            bk_sb = consts.tile([P, ET], F32)
            nc.gpsimd.dma_start(bk_sb[:], bk[:].rearrange("(t p) -> p t", p=P))

            for dt in range(DT):
                dma(xc[1][:, dt, :], xT[dt * P:(dt + 1) * P, 512:1024])
            for dt in range(DT):
                dma(wv_sb[:, dt, :], wvT[dt * P:(dt + 1) * P, :])
            for dt in range(DT):
                dma(wq_sb[:, dt, :], wqT[dt * P:(dt + 1) * P, :])

            # chunk-0 K projection dt-outer/et-inner: the first 8 matmuls only
            # need wk[dt0]+x0[dt0], so the PE starts as soon as ~384KB landed.
            with tc.tile_pool(name="p8", bufs=8, space="PSUM") as p8:
                psk0 = [p8.tile([P, 512], F32, tag="p8", name=f"psk0_{et}")
                        for et in range(ET)]
                for dt in range(DT):
                    for et in range(ET):
                        nc.tensor.matmul(psk0[et][:],
                                         wk_sb[:, dt, et * P:(et + 1) * P],
                                         xc[0][:, dt, :], start=(dt == 0),
                                         stop=(dt == DT - 1))
                for et in range(ET):
                    nc.vector.tensor_scalar_add(kown[:, et, 0:512],
                                                psk0[et][:], bk_sb[:, et:et + 1])

            with tc.tile_pool(name="pp", bufs=3, space="PSUM") as pp:
                # K chunk 1
                for et in range(ET):
                    psk = pp.tile([P, 512], F32, tag="pp")
                    for dt in range(DT):
                        nc.tensor.matmul(psk[:], wk_sb[:, dt, et * P:(et + 1) * P],
                                         xc[1][:, dt, :], start=(dt == 0),
                                         stop=(dt == DT - 1))
                    nc.vector.tensor_scalar_add(kown[:, et, 512:1024],
                                                psk[:], bk_sb[:, et:et + 1])
                # K-half complete -> bounce out + AllGather (overlaps V/Q)
                for et in range(ET):
                    dma(kb_in[:, et, :], kown[:, et, :])
                nc.gpsimd.collective_compute(
                    "AllGather", mybir.AluOpType.bypass, replica_groups=PAIRS,
                    ins=[kb_in[:].opt()], outs=[kb_out[:].opt()])
                for r in range(2):
                    for et in range(ET):
                        dma(kT[:, et, r * NQ:(r + 1) * NQ], kb_out[r, :, et, :])

                # bv broadcast to [P, D] via ones.T @ bv (K=1 matmul)
                for ec in range(EC):
                    pstmp = pp.tile([P, 512], F32, tag="pp")
                    nc.tensor.matmul(pstmp[:], ones[:],
                                     bv_sb[:, ec * 512:(ec + 1) * 512],
                                     start=True, stop=True)
                    nc.any.tensor_copy(bv_bcast[:, ec * 512:(ec + 1) * 512], pstmp[:])

                # V projection (own half)
                for hc in range(HC):
                    for sb_i in range(4):
                        jg = hc * 4 + sb_i
                        for ec in range(EC):
                            psv = pp.tile([P, 512], F32, tag="pp")
                            for dt in range(DT):
                                nc.tensor.matmul(psv[:],
                                                 xc[hc][:, dt, sb_i * P:(sb_i + 1) * P],
                                                 wv_sb[:, dt, ec * 512:(ec + 1) * 512],
                                                 start=(dt == 0), stop=(dt == DT - 1))
                            nc.any.tensor_copy(vown[:, jg, ec * 512:(ec + 1) * 512],
                                               psv[:])
                # V-half complete -> bounce out + AllGather (overlaps Q)
                for jg in range(HB):
                    dma(vb_in[:, jg, :], vown[:, jg, :])
                nc.gpsimd.collective_compute(
                    "AllGather", mybir.AluOpType.bypass, replica_groups=PAIRS,
                    ins=[vb_in[:].opt()], outs=[vb_out[:].opt()])
                for r in range(2):
                    for jg in range(HB):
                        dma(vN[:, r * HB + jg, :], vb_out[r, :, jg, :])

                # Q projection
                for hc in range(HC):
                    for et in range(ET):
                        psq = pp.tile([P, 512], F32, tag="pp")
                        for dt in range(DT):
                            nc.tensor.matmul(psq[:],
                                             wq_sb[:, dt, et * P:(et + 1) * P],
                                             xc[hc][:, dt, :], start=(dt == 0),
                                             stop=(dt == DT - 1))
                        nc.vector.tensor_scalar_add(
                            qT[:, et, hc * 512:(hc + 1) * 512],
                            psq[:], bq_sb[:, et:et + 1])

        # ---- Phase 2: attention ----
        with tc.tile_pool(name="attn", bufs=1) as attnp, \
             tc.tile_pool(name="scps", bufs=2, space="PSUM") as scps, \
             tc.tile_pool(name="rsps", bufs=2, space="PSUM") as rsps, \
             tc.tile_pool(name="outps", bufs=2, space="PSUM") as outps, \
             tc.tile_pool(name="rsdram", bufs=2, space="DRAM") as rsdram, \
             tc.tile_pool(name="epi", bufs=2) as epip:
            attnT = attnp.tile([P, SB, 512], BF)     # [j-part, j-tile, i] one chunk
            for g in range(IG):
                # scores (transposed) + exp + rowsum
                rs = rsps.tile([1, 512], F32, tag="rs", name=f"rs{g}")
                for jt in range(SB):
                    sc_ps = scps.tile([P, 512], F32, tag="scps")
                    for et in range(ET):
                        nc.tensor.matmul(sc_ps[:],
                                         kT[:, et, jt * P:(jt + 1) * P],
                                         qT[:, et, g * 512:(g + 1) * 512],
                                         start=(et == 0), stop=(et == ET - 1))
                    nc.scalar.activation(attnT[:, jt, :], sc_ps[:], AF.Exp,
                                         scale=INV_SQRT_D)
                    nc.tensor.matmul(rs[:], onesb[:], attnT[:, jt, :],
                                     start=(jt == 0), stop=(jt == SB - 1))
                # rowsums -> per-partition layout [128, 4] via DRAM bounce
                # (SBUF partition data can't be re-partitioned by an AP alone)
                rs_sb = epip.tile([1, 512], F32, tag="rs_sb")
                nc.vector.tensor_copy(rs_sb[:], rs[:])
                rs_d = rsdram.tile([1, 512], F32, tag="rs_d")
                nc.sync.dma_start(rs_d[:], rs_sb[:])
                rsT = epip.tile([P, 4], F32, tag="rsT")
                nc.sync.dma_start(
                    rsT[:], rs_d[:].rearrange("one (b p) -> p (one b)", p=P))
                invs = epip.tile([P, 4], F32, tag="invs")
                nc.vector.reciprocal(invs[:], rsT[:])
                # output matmuls + epilogue per 128-query block
                for ib in range(4):
                    i0 = ib * P
                    ops = [outps.tile([P, 512], F32, tag=f"outps{ec}",
                                      name=f"ops{g}_{ib}_{ec}") for ec in range(EC)]
                    for jt in range(SB):
                        for ec in range(EC):
                            nc.tensor.matmul(ops[ec][:],
                                             attnT[:, jt, i0:i0 + P],
                                             vN[:, jt, ec * 512:(ec + 1) * 512],
                                             start=(jt == 0), stop=(jt == SB - 1))
                    out_sb = epip.tile([P, D], F32, tag="out_sb")
                    for ec in range(EC):
                        nc.vector.tensor_scalar_mul(out_sb[:, ec * 512:(ec + 1) * 512],
                                                    ops[ec][:], invs[:, ib:ib + 1])
                    nc.vector.tensor_add(out_sb[:], out_sb[:], bv_bcast[:])
                    r0 = g * 512 + i0
                    dma(out[r0:r0 + P, :], out_sb[:])

    nc.compile()
    return nc


def make_in_maps(x, Wq, bq, Wk, bk, Wv, bv):
    import ml_dtypes
    BF = ml_dtypes.bfloat16
    x = np.asarray(x, np.float32)
    wqT = np.ascontiguousarray(np.asarray(Wq, np.float32).T.astype(BF))
    wkT = np.ascontiguousarray(np.asarray(Wk, np.float32).T.astype(BF))
    wvT = np.ascontiguousarray(np.asarray(Wv, np.float32).T.astype(BF))
    bq = np.ascontiguousarray(np.asarray(bq, np.float32))
    bk = np.ascontiguousarray(np.asarray(bk, np.float32))
    bv = np.ascontiguousarray(np.asarray(bv, np.float32))
    in_maps = []
    for c in range(NCORES):
        b, h = c // 2, c % 2
        xb = x[b]
        in_maps.append({
            "xT": np.ascontiguousarray(xb[h * NQ:(h + 1) * NQ].T.astype(BF)),
            "wqT": wqT, "wkT": wkT, "wvT": wvT,
            "bq": bq, "bk": bk, "bv": bv,
        })
    return in_maps


def get_nc():
    if "nc" not in _CACHE:
        _CACHE["nc"] = build_nc()
    return _CACHE["nc"]


def kernel(x, Wq, bq, Wk, bk, Wv, bv):
    from concourse.bass_utils import run_bass_kernel_spmd
    nc = get_nc()
    in_maps = make_in_maps(x, Wq, bq, Wk, bk, Wv, bv)
    res = run_bass_kernel_spmd(nc, in_maps, core_ids=list(range(NCORES)))
    out = np.empty((B, S, D), np.float32)
    for c in range(NCORES):
        b, h = c // 2, c % 2
        out[b, h * NQ:(h + 1) * NQ] = res.results[c]["out"]
    return out
